# revision 22
# baseline (speedup 1.0000x reference)
"""NeuromorphicBrainZone Trainium2 kernel (8 NeuronCores, Bass/Tile).

Math (per reference):
    x2 = x.reshape(T, D)                                     # T=1024, D=512
    zone[t, j] = b_in[j] - mean_d |x2[t, d] - W_in[j, d]|    # N=2048
    spikes     = sigmoid(SURR_BETA * (zone - v_th))
    out[t, m]  = b_out[m] - mean_j |spikes[t, j] - W_out[m, j]|

Sharding: layer-1 neuron dim j sharded 8 ways (JC=256 j per core, all
tokens). Layer 2 reduces over j, so each core computes partials over its
local j for ALL (t, m); a single fp16 ReduceScatter(add) completes the
j-reduction and leaves each core a 64-row m-shard of the output.

Algorithm (weight quantization -> fp8 DoubleRow matmuls):
Weights are quantized to K levels theta_k[p] (shared across reduce-dim
blocks at the same partition index p; per-partition quantiles, rounded
to fp8). Three elementwise forms per level:
    max  (DVE):  |x - wq| = 2*max(x, wq) - x - wq        (exact in fp8)
    relu (ACT):  |x - wq| = 2*relu(x - wq) - x + wq
    zero (free): |s - wq| = s - wq        when s >= 0 >= wq  (layer 2!)
With S_k[r, j] = 2 * 1[quant(w_jr) = k],
    sum_r 2*f_k(x_tr) selected = sum_k (S_k^T @ M_k)[j, t].
Per-(t,r) elementwise work is K tiles for L1 (split DVE/ACT) and only
the positive levels for L2 (spikes are nonnegative, so every negative
level is handled by the sign matrix alone). The reduction runs as dense
fp8 DoubleRow matmuls on the PE (two 128-deep contraction slices per
instruction at 0.5 cycles/row). The -+sum_r x term is a +-1 lhsT
DoubleRow matmul streaming x (or spikes); the -+sum_r wq term folds
into the evacuation bias on the host.
"""

import sys

sys.path.insert(0, "/opt/trn_rl_repo")

from contextlib import ExitStack

import numpy as np

import concourse.bass as bass
import concourse.bacc as bacc
import concourse.mybir as mybir
import concourse.tile as tile

SURR_BETA = 4.0
K = 16                      # quantization levels per weight column
ACT1_KS = frozenset((10, 11, 12, 13, 14, 15))  # L1 levels on ACT (relu form)
NZ2 = K // 2                # L2: levels 0..NZ2-1 clamped <= 0 ("zero" form)
N_CORES = 8
T, D, N, M = 1024, 512, 2048, 512
JC = N // N_CORES           # local neurons (L1 outputs per core)
MS = M // N_CORES           # final output m-shard rows per core
N_DBLK = D // 128           # 4
N_JBLK = JC // 128          # 2
N_MBLK = M // 128           # 4
CH = 512                    # matmul free-dim chunk (one PSUM bank)
N_CH = T // CH              # 2


def build_kernel():
    f32 = mybir.dt.float32
    f16 = mybir.dt.float16
    f8 = mybir.dt.float8e4
    DR = mybir.MatmulPerfMode.DoubleRow
    Act = mybir.ActivationFunctionType

    nc = bacc.Bacc("TRN2", target_bir_lowering=False, debug=False,
                   num_devices=N_CORES)

    xp_d = nc.dram_tensor("xp", [128, N_DBLK * T], f8, kind="ExternalInput")
    th1p_d = nc.dram_tensor("th1p", [128, K], f32, kind="ExternalInput")
    th1n_d = nc.dram_tensor("th1n", [128, K], f32, kind="ExternalInput")
    # s1 rows (k*2+dbp)*128 + p; cols two*JC + j  (two = db within pair)
    s1_d = nc.dram_tensor("s1", [K * 2 * 128, 2 * JC], f8,
                          kind="ExternalInput")
    beta_d = nc.dram_tensor("beta", [128, N_JBLK], f32, kind="ExternalInput")
    th2p_d = nc.dram_tensor("th2p", [128, K], f32, kind="ExternalInput")
    # s2 rows (k-NZ2)*128 + p; cols jb*M + m  (positive levels only)
    s2_d = nc.dram_tensor("s2", [(K - NZ2) * 128, N_JBLK * M], f8,
                          kind="ExternalInput")
    # sigma2 rows p; cols jb*M + m; +1 zero-form / -1 otherwise
    sg2_d = nc.dram_tensor("sg2", [128, N_JBLK * M], f8, kind="ExternalInput")
    bo_d = nc.dram_tensor("bo", [128, N_MBLK], f32, kind="ExternalInput")
    out_d = nc.dram_tensor("out", [MS, T], f16, kind="ExternalOutput")

    with tile.TileContext(nc) as tc, ExitStack() as ctx:
        cpool = ctx.enter_context(tc.tile_pool(name="const", bufs=1))
        mpool = ctx.enter_context(tc.tile_pool(name="m", bufs=8))
        ppool = ctx.enter_context(tc.tile_pool(name="psum", bufs=4,
                                               space="PSUM"))
        dpool = ctx.enter_context(tc.tile_pool(name="dram", bufs=1,
                                               space="DRAM"))

        # Warm up the collective pipeline first: pays the one-time CC mesh
        # init (~30us) concurrently with compute instead of on the tail.
        wu_in = dpool.tile([8, 4], f32, tag="wu_in", name="wu_in")
        wu_out = dpool.tile([8, 4], f32, tag="wu_out", name="wu_out")
        nc.gpsimd.collective_compute(
            "AllToAll", mybir.AluOpType.bypass,
            replica_groups=[list(range(N_CORES))],
            ins=[wu_in.opt()], outs=[wu_out.opt()])

        def load(name, src, shape, dtype):
            t = cpool.tile(shape, dtype, tag=name, name=name)
            nc.sync.dma_start(t[:], src)
            return t

        xp = cpool.tile([128, N_DBLK * T], f8, tag="xp", name="xp")
        for db in range(N_DBLK):
            nc.sync.dma_start(xp[:, db * T:(db + 1) * T],
                              xp_d[:, db * T:(db + 1) * T])
        th1p = load("th1p", th1p_d[:, :], [128, K], f32)
        th1n = load("th1n", th1n_d[:, :], [128, K], f32)
        s1 = [[load(f"s1_{k}_{dbp}",
                    s1_d[(k * 2 + dbp) * 128:(k * 2 + dbp + 1) * 128, :],
                    [128, 2 * JC], f8) for dbp in range(2)]
              for k in range(K)]
        beta = load("beta", beta_d[:, :], [128, N_JBLK], f32)
        th2p = load("th2p", th2p_d[:, :], [128, K], f32)
        s2 = [load(f"s2_{k}", s2_d[(k - NZ2) * 128:(k - NZ2 + 1) * 128, :],
                   [128, N_JBLK * M], f8) for k in range(NZ2, K)]
        sg2 = load("sg2", sg2_d[:, :], [128, N_JBLK * M], f8)
        bo = load("bo", bo_d[:, :], [128, N_MBLK], f32)

        negdr = cpool.tile([128, 256], f8, tag="negdr", name="negdr")
        nc.vector.memset(negdr[:], -1.0)
        negw = negdr[:].rearrange("p (two o) -> p two o", two=2)
        spikes = cpool.tile([128, N_JBLK * T], f8, tag="spk", name="spk")
        spk3 = spikes[:].rearrange("p (jb t) -> p jb t", t=T)
        xp3 = xp[:].rearrange("p (db t) -> p db t", t=T)
        m2s = {k: cpool.tile([128, N_JBLK * T], f8, tag=f"m2_{k}",
                             name=f"m2_{k}") for k in range(NZ2, K)}
        partial = [cpool.tile([128, T], f16, tag=f"par{mb}", name=f"par{mb}")
                   for mb in range(N_MBLK)]

        # ---- layer 1: ps1[jb][j, t] = 2*sum_d f(x, wq) - sum_d x ----
        ps1 = [ppool.tile([128, T], f32, tag="ps", name=f"ps1_{jb}")
               for jb in range(N_JBLK)]
        k_order = list(range(K))
        for ki, k in enumerate(k_order):
            m1 = mpool.tile([128, N_DBLK * T], f8, tag="m1", name="m1")
            if k in ACT1_KS:
                nc.scalar.activation(m1[:], xp[:], Act.Relu,
                                     bias=th1n[:, k:k + 1], scale=1.0)
            else:
                nc.vector.tensor_scalar(m1[:], xp[:], th1p[:, k:k + 1], None,
                                        op0=mybir.AluOpType.max)
            m13 = m1[:].rearrange("p (db t) -> p db t", t=T)
            for dbp in range(2):
                for jb in range(N_JBLK):
                    for c in range(N_CH):
                        nc.tensor.matmul(
                            ps1[jb][:, c * CH:(c + 1) * CH],
                            s1[k][dbp][:].rearrange(
                                "p (two o) -> p two o", two=2)
                            [:, :, jb * 128:(jb + 1) * 128],
                            m13[:, dbp * 2:dbp * 2 + 2,
                                c * CH:(c + 1) * CH],
                            start=(ki == 0 and dbp == 0), stop=False,
                            perf_mode=DR)
        for jb in range(N_JBLK):
            for dbp in range(2):
                for c in range(N_CH):
                    nc.tensor.matmul(
                        ps1[jb][:, c * CH:(c + 1) * CH], negw,
                        xp3[:, dbp * 2:dbp * 2 + 2, c * CH:(c + 1) * CH],
                        start=False, stop=(dbp == 1 and c == N_CH - 1),
                        perf_mode=DR)
            nc.scalar.activation(spikes[:, jb * T:(jb + 1) * T], ps1[jb][:],
                                 Act.Sigmoid, bias=beta[:, jb:jb + 1],
                                 scale=-SURR_BETA / D)
            # m2 tiles for this jb half as soon as its spikes exist
            # (positive levels only; negative levels are the free zero form)
            for k in range(NZ2, K):
                nc.vector.tensor_scalar(
                    m2s[k][:, jb * T:(jb + 1) * T],
                    spikes[:, jb * T:(jb + 1) * T],
                    th2p[:, k:k + 1], None, op0=mybir.AluOpType.max)

        # ---- layer 2: per-mblock accumulate, single fp16 RS ----
        cin = dpool.tile([M, T], f16, tag="cin", name="cin")
        cout = dpool.tile([M, T], f16, tag="cout", name="cout")
        for mb in range(N_MBLK):
            ps2 = ppool.tile([128, T], f32, tag="ps", name=f"ps2_{mb}")
            for k in range(NZ2, K):
                for c in range(N_CH):
                    nc.tensor.matmul(
                        ps2[:, c * CH:(c + 1) * CH],
                        s2[k - NZ2][:].rearrange("p (jb m) -> p jb m", m=M)
                        [:, :, mb * 128:(mb + 1) * 128],
                        m2s[k][:].rearrange("p (jb t) -> p jb t", t=T)
                        [:, :, c * CH:(c + 1) * CH],
                        start=(k == NZ2), stop=False, perf_mode=DR)
            for c in range(N_CH):
                nc.tensor.matmul(
                    ps2[:, c * CH:(c + 1) * CH],
                    sg2[:].rearrange("p (jb m) -> p jb m", m=M)
                    [:, :, mb * 128:(mb + 1) * 128],
                    spk3[:, :, c * CH:(c + 1) * CH],
                    start=False, stop=(c == N_CH - 1), perf_mode=DR)
            nc.scalar.activation(partial[mb][:], ps2[:], Act.Identity,
                                 bias=bo[:, mb:mb + 1], scale=-1.0 / N)
            nc.sync.dma_start(cin[mb * 128:(mb + 1) * 128, :],
                              partial[mb][:])
        nc.gpsimd.collective_compute(
            "AllToAll", mybir.AluOpType.bypass,
            replica_groups=[list(range(N_CORES))],
            ins=[cin.opt()], outs=[cout.opt()])
        rx = cpool.tile([MS, N_CORES * T], f16, tag="rx", name="rx")
        acc = cpool.tile([MS, T], f16, tag="acc", name="acc")
        T2 = T // 2
        for h in range(2):
            hb = h * N_CORES * T2          # half's contiguous chunk region
            for i in range(N_CORES):
                nc.sync.dma_start(
                    rx[:, hb + i * T2:hb + (i + 1) * T2],
                    cout[i * MS:(i + 1) * MS, h * T2:(h + 1) * T2])
            # tree: 8 chunks -> 4 -> 2 -> 1 in three wide adds
            nc.vector.tensor_tensor(
                rx[:, hb:hb + 4 * T2], rx[:, hb:hb + 4 * T2],
                rx[:, hb + 4 * T2:hb + 8 * T2], op=mybir.AluOpType.add)
            nc.vector.tensor_tensor(
                rx[:, hb:hb + 2 * T2], rx[:, hb:hb + 2 * T2],
                rx[:, hb + 2 * T2:hb + 4 * T2], op=mybir.AluOpType.add)
            nc.vector.tensor_tensor(
                acc[:, h * T2:(h + 1) * T2], rx[:, hb:hb + T2],
                rx[:, hb + T2:hb + 2 * T2], op=mybir.AluOpType.add)
            nc.sync.dma_start(out_d[:, h * T2:(h + 1) * T2],
                              acc[:, h * T2:(h + 1) * T2])

    nc.compile()
    return nc


def _quant_shared(W, nblk, dt, clamp_nonpos_below=None):
    """W [n_out, n_red] -> per-partition-index levels shared across the
    n_red/128 blocks. Levels sorted ascending. Returns lv [128, K] (f32,
    dt-representable), idx [n_out, n_red], Wq [n_out, n_red]."""
    n_out, n_red = W.shape
    Wp = W.reshape(n_out, nblk, 128)
    qs = (np.arange(K, dtype=np.float64) + 0.5) / K
    lv = np.quantile(Wp.transpose(2, 0, 1).reshape(128, -1), qs, axis=1).T
    if clamp_nonpos_below is not None:
        lv[:, :clamp_nonpos_below] = np.minimum(lv[:, :clamp_nonpos_below], 0)
    lv = lv.astype(dt).astype(np.float32)                    # [128, K]
    if clamp_nonpos_below is not None:
        lv[:, :clamp_nonpos_below] = np.minimum(lv[:, :clamp_nonpos_below], 0)
    idx = np.abs(Wp[:, :, :, None]
                 - lv[None, None, :, :]).argmin(axis=3)      # [n_out,blk,128]
    Wq = lv[np.arange(128)[None, None, :], idx]
    return lv, idx.reshape(n_out, n_red), Wq.reshape(n_out, n_red)


def prep_inputs(x, W_in, b_in, W_out, b_out, v_th, n_cores=N_CORES):
    """Host-side prep: pack/transpose + weight quantization per core."""
    import ml_dtypes

    f8 = ml_dtypes.float8_e4m3

    x2 = np.asarray(x, np.float32).reshape(T, D)
    x8 = np.ascontiguousarray(x2.T).astype(f8)               # [D, T]
    xp = np.ascontiguousarray(
        x8.reshape(N_DBLK, 128, T).transpose(1, 0, 2).reshape(128, N_DBLK * T))

    # L1 quantization is global (same W_in for every core).
    lv1, idx1, Wq1 = _quant_shared(np.asarray(W_in, np.float32), N_DBLK, f8)
    th1p = np.ascontiguousarray(lv1)
    th1n = np.ascontiguousarray(-lv1)
    # eta: +1 for relu-form (ACT) levels, -1 for max-form (DVE) levels
    eta1 = np.where(np.isin(np.arange(K), list(ACT1_KS)), 1.0, -1.0)
    etaw1 = (eta1[idx1] * Wq1).astype(np.float64).sum(axis=1)  # [N]
    beta_full = (SURR_BETA * (np.asarray(b_in, np.float64)
                              - np.asarray(v_th, np.float64)
                              - etaw1 / D)).astype(np.float32)

    in_maps = []
    for c in range(n_cores):
        sl = slice(c * JC, (c + 1) * JC)
        idx1_loc = idx1[sl, :]                               # [JC, D]
        # s1 rows (k*2+dbp)*128+p, cols two*JC + j
        s1 = np.zeros((K, 2, 128, 2 * JC), np.float32)
        for db in range(N_DBLK):
            dbp, two = divmod(db, 2)
            blk = idx1_loc[:, db * 128:(db + 1) * 128].T     # [128(p), JC]
            for k in range(K):
                s1[k, dbp, :, two * JC:(two + 1) * JC] = 2.0 * (blk == k)
        s1 = s1.reshape(K * 2 * 128, 2 * JC).astype(f8)

        beta = np.ascontiguousarray(
            beta_full[sl].reshape(N_JBLK, 128).T).astype(np.float32)

        # L2: quantize this core's W_out column slice. Levels 0..NZ2-1 are
        # clamped <= 0 and handled by the free zero form (spikes >= 0).
        W2c = np.asarray(W_out, np.float32)[:, sl]           # [M, JC]
        lv2, idx2, Wq2 = _quant_shared(W2c, N_JBLK, f8,
                                       clamp_nonpos_below=NZ2)
        th2p = np.ascontiguousarray(lv2)
        # s2 rows (k-NZ2)*128+p, cols jb*M + m  (positive levels only)
        s2 = np.zeros((K - NZ2, 128, N_JBLK * M), np.float32)
        sg2 = np.zeros((128, N_JBLK * M), np.float32)
        for jb in range(N_JBLK):
            blk = idx2[:, jb * 128:(jb + 1) * 128]           # [M, 128(p)]
            for k in range(NZ2, K):
                s2[k - NZ2, :, jb * M:(jb + 1) * M] = 2.0 * (blk == k).T
            sg2[:, jb * M:(jb + 1) * M] = np.where(
                (blk < NZ2) | np.isin(blk, list(ACT2_KS)), 1.0, -1.0).T
        s2 = s2.reshape((K - NZ2) * 128, N_JBLK * M).astype(f8)
        sg2 = sg2.astype(f8)

        # wq coefficient is -1 for both max and zero forms
        etaw2 = (-Wq2).astype(np.float64).sum(axis=1)        # [M]
        bo_full = (np.asarray(b_out, np.float64) / n_cores
                   - etaw2 / N).astype(np.float32)
        bo = np.ascontiguousarray(
            bo_full.reshape(N_MBLK, 128).T).astype(np.float32)

        in_maps.append({
            "xp": xp, "th1p": th1p, "th1n": th1n, "s1": s1, "beta": beta,
            "th2p": th2p, "s2": s2, "sg2": sg2, "bo": bo,
        })
    return in_maps


_NC_CACHE = {}


def _get_nc():
    if "nc" not in _NC_CACHE:
        _NC_CACHE["nc"] = build_kernel()
    return _NC_CACHE["nc"]


def run_on_hw(inputs, trace=False, tmpdir=None):
    """Run on the 8 NeuronCores; returns (full_output, BassKernelResults)."""
    from concourse.bass_utils import run_bass_kernel_spmd

    nc = _get_nc()
    in_maps = prep_inputs(**inputs)
    res = run_bass_kernel_spmd(nc, in_maps, core_ids=list(range(N_CORES)),
                               trace=trace, tmpdir=tmpdir)
    B, S, D_model = inputs["x"].shape
    full = np.empty((M, T), np.float32)
    for c in range(N_CORES):
        full[c * MS:(c + 1) * MS, :] = \
            np.asarray(res.results[c]["out"], np.float32)
    out = np.ascontiguousarray(full.T).reshape(B, S, D_model)
    return out.astype(np.float32), res


def kernel(x, W_in, b_in, W_out, b_out, v_th):
    out, _ = run_on_hw(dict(x=x, W_in=W_in, b_in=b_in, W_out=W_out,
                            b_out=b_out, v_th=v_th))
    return out


# revision 23
# speedup vs baseline: 1.0867x; 1.0867x over previous
"""NeuromorphicBrainZone Trainium2 kernel (8 NeuronCores, Bass/Tile).

Math (per reference):
    x2 = x.reshape(T, D)                                     # T=1024, D=512
    zone[t, j] = b_in[j] - mean_d |x2[t, d] - W_in[j, d]|    # N=2048
    spikes     = sigmoid(SURR_BETA * (zone - v_th))
    out[t, m]  = b_out[m] - mean_j |spikes[t, j] - W_out[m, j]|

Sharding: layer-1 neuron dim j sharded 8 ways (JC=256 j per core, all
tokens). Layer 2 reduces over j, so each core computes partials over its
local j for ALL (t, m); a single fp16 ReduceScatter(add) completes the
j-reduction and leaves each core a 64-row m-shard of the output.

Algorithm (weight quantization -> fp8 DoubleRow matmuls):
Weights are quantized to K levels theta_k[p] (shared across reduce-dim
blocks at the same partition index p; per-partition quantiles, rounded
to fp8). Three elementwise forms per level:
    max  (DVE):  |x - wq| = 2*max(x, wq) - x - wq        (exact in fp8)
    relu (ACT):  |x - wq| = 2*relu(x - wq) - x + wq
    zero (free): |s - wq| = s - wq        when s >= 0 >= wq  (layer 2!)
With S_k[r, j] = 2 * 1[quant(w_jr) = k],
    sum_r 2*f_k(x_tr) selected = sum_k (S_k^T @ M_k)[j, t].
Per-(t,r) elementwise work is K tiles for L1 (split DVE/ACT) and only
the positive levels for L2 (spikes are nonnegative, so every negative
level is handled by the sign matrix alone). The reduction runs as dense
fp8 DoubleRow matmuls on the PE (two 128-deep contraction slices per
instruction at 0.5 cycles/row). The -+sum_r x term is a +-1 lhsT
DoubleRow matmul streaming x (or spikes); the -+sum_r wq term folds
into the evacuation bias on the host.
"""

import sys

sys.path.insert(0, "/opt/trn_rl_repo")

from contextlib import ExitStack

import numpy as np

import concourse.bass as bass
import concourse.bacc as bacc
import concourse.mybir as mybir
import concourse.tile as tile

SURR_BETA = 4.0
K = 16                      # quantization levels per weight column
ACT1_KS = frozenset((10, 11, 12, 13, 14, 15))  # L1 levels on ACT (relu form)
NZ2 = K // 2                # L2: levels 0..NZ2-1 clamped <= 0 ("zero" form)
N_CORES = 8
T, D, N, M = 1024, 512, 2048, 512
JC = N // N_CORES           # local neurons (L1 outputs per core)
MS = M // N_CORES           # final output m-shard rows per core
N_DBLK = D // 128           # 4
N_JBLK = JC // 128          # 2
N_MBLK = M // 128           # 4
CH = 512                    # matmul free-dim chunk (one PSUM bank)
N_CH = T // CH              # 2


def build_kernel():
    f32 = mybir.dt.float32
    f16 = mybir.dt.float16
    f8 = mybir.dt.float8e4
    DR = mybir.MatmulPerfMode.DoubleRow
    Act = mybir.ActivationFunctionType

    nc = bacc.Bacc("TRN2", target_bir_lowering=False, debug=False,
                   num_devices=N_CORES)

    xp_d = nc.dram_tensor("xp", [128, N_DBLK * T], f8, kind="ExternalInput")
    th1p_d = nc.dram_tensor("th1p", [128, K], f32, kind="ExternalInput")
    th1n_d = nc.dram_tensor("th1n", [128, K], f32, kind="ExternalInput")
    # s1 rows (k*2+dbp)*128 + p; cols two*JC + j  (two = db within pair)
    s1_d = nc.dram_tensor("s1", [K * 2 * 128, 2 * JC], f8,
                          kind="ExternalInput")
    beta_d = nc.dram_tensor("beta", [128, N_JBLK], f32, kind="ExternalInput")
    th2p_d = nc.dram_tensor("th2p", [128, K], f32, kind="ExternalInput")
    # s2 rows (k-NZ2)*128 + p; cols jb*M + m  (positive levels only)
    s2_d = nc.dram_tensor("s2", [(K - NZ2) * 128, N_JBLK * M], f8,
                          kind="ExternalInput")
    # sigma2 rows p; cols jb*M + m; +1 zero-form / -1 otherwise
    sg2_d = nc.dram_tensor("sg2", [128, N_JBLK * M], f8, kind="ExternalInput")
    bo_d = nc.dram_tensor("bo", [128, N_MBLK], f32, kind="ExternalInput")
    out_d = nc.dram_tensor("out", [MS, T], f16, kind="ExternalOutput")

    with tile.TileContext(nc) as tc, ExitStack() as ctx:
        cpool = ctx.enter_context(tc.tile_pool(name="const", bufs=1))
        mpool = ctx.enter_context(tc.tile_pool(name="m", bufs=8))
        ppool = ctx.enter_context(tc.tile_pool(name="psum", bufs=4,
                                               space="PSUM"))
        dpool = ctx.enter_context(tc.tile_pool(name="dram", bufs=1,
                                               space="DRAM"))

        # Warm up the collective pipeline first: pays the one-time CC mesh
        # init (~30us) concurrently with compute instead of on the tail.
        wu_in = dpool.tile([1, 4], f32, tag="wu_in", name="wu_in")
        wu_out = dpool.tile([8, 4], f32, tag="wu_out", name="wu_out")
        nc.gpsimd.collective_compute(
            "AllGather", mybir.AluOpType.bypass,
            replica_groups=[list(range(N_CORES))],
            ins=[wu_in.opt()], outs=[wu_out.opt()])

        def load(name, src, shape, dtype):
            t = cpool.tile(shape, dtype, tag=name, name=name)
            nc.sync.dma_start(t[:], src)
            return t

        xp = cpool.tile([128, N_DBLK * T], f8, tag="xp", name="xp")
        for db in range(N_DBLK):
            nc.sync.dma_start(xp[:, db * T:(db + 1) * T],
                              xp_d[:, db * T:(db + 1) * T])
        th1p = load("th1p", th1p_d[:, :], [128, K], f32)
        th1n = load("th1n", th1n_d[:, :], [128, K], f32)
        s1 = [[load(f"s1_{k}_{dbp}",
                    s1_d[(k * 2 + dbp) * 128:(k * 2 + dbp + 1) * 128, :],
                    [128, 2 * JC], f8) for dbp in range(2)]
              for k in range(K)]
        beta = load("beta", beta_d[:, :], [128, N_JBLK], f32)
        th2p = load("th2p", th2p_d[:, :], [128, K], f32)
        s2 = [load(f"s2_{k}", s2_d[(k - NZ2) * 128:(k - NZ2 + 1) * 128, :],
                   [128, N_JBLK * M], f8) for k in range(NZ2, K)]
        sg2 = load("sg2", sg2_d[:, :], [128, N_JBLK * M], f8)
        bo = load("bo", bo_d[:, :], [128, N_MBLK], f32)

        negdr = cpool.tile([128, 256], f8, tag="negdr", name="negdr")
        nc.vector.memset(negdr[:], -1.0)
        negw = negdr[:].rearrange("p (two o) -> p two o", two=2)
        spikes = cpool.tile([128, N_JBLK * T], f8, tag="spk", name="spk")
        spk3 = spikes[:].rearrange("p (jb t) -> p jb t", t=T)
        xp3 = xp[:].rearrange("p (db t) -> p db t", t=T)
        m2s = {k: cpool.tile([128, N_JBLK * T], f8, tag=f"m2_{k}",
                             name=f"m2_{k}") for k in range(NZ2, K)}
        partial = [cpool.tile([128, T], f16, tag=f"par{mb}", name=f"par{mb}")
                   for mb in range(N_MBLK)]

        # ---- layer 1: ps1[jb][j, t] = 2*sum_d f(x, wq) - sum_d x ----
        ps1 = [ppool.tile([128, T], f32, tag="ps", name=f"ps1_{jb}")
               for jb in range(N_JBLK)]
        k_order = list(range(K))
        for ki, k in enumerate(k_order):
            m1 = mpool.tile([128, N_DBLK * T], f8, tag="m1", name="m1")
            if k in ACT1_KS:
                nc.scalar.activation(m1[:], xp[:], Act.Relu,
                                     bias=th1n[:, k:k + 1], scale=1.0)
            else:
                nc.vector.tensor_scalar(m1[:], xp[:], th1p[:, k:k + 1], None,
                                        op0=mybir.AluOpType.max)
            m13 = m1[:].rearrange("p (db t) -> p db t", t=T)
            for dbp in range(2):
                for jb in range(N_JBLK):
                    for c in range(N_CH):
                        nc.tensor.matmul(
                            ps1[jb][:, c * CH:(c + 1) * CH],
                            s1[k][dbp][:].rearrange(
                                "p (two o) -> p two o", two=2)
                            [:, :, jb * 128:(jb + 1) * 128],
                            m13[:, dbp * 2:dbp * 2 + 2,
                                c * CH:(c + 1) * CH],
                            start=(ki == 0 and dbp == 0), stop=False,
                            perf_mode=DR)
        for jb in range(N_JBLK):
            for dbp in range(2):
                for c in range(N_CH):
                    nc.tensor.matmul(
                        ps1[jb][:, c * CH:(c + 1) * CH], negw,
                        xp3[:, dbp * 2:dbp * 2 + 2, c * CH:(c + 1) * CH],
                        start=False, stop=(dbp == 1 and c == N_CH - 1),
                        perf_mode=DR)
            nc.scalar.activation(spikes[:, jb * T:(jb + 1) * T], ps1[jb][:],
                                 Act.Sigmoid, bias=beta[:, jb:jb + 1],
                                 scale=-SURR_BETA / D)
            # m2 tiles for this jb half as soon as its spikes exist
            # (positive levels only; negative levels are the free zero form)
            for k in range(NZ2, K):
                nc.vector.tensor_scalar(
                    m2s[k][:, jb * T:(jb + 1) * T],
                    spikes[:, jb * T:(jb + 1) * T],
                    th2p[:, k:k + 1], None, op0=mybir.AluOpType.max)

        # ---- layer 2: per-mblock accumulate, single fp16 RS ----
        cin = dpool.tile([M, T], f16, tag="cin", name="cin")
        cout = dpool.tile([M, T], f16, tag="cout", name="cout")
        for mb in range(N_MBLK):
            ps2 = ppool.tile([128, T], f32, tag="ps", name=f"ps2_{mb}")
            for k in range(NZ2, K):
                for c in range(N_CH):
                    nc.tensor.matmul(
                        ps2[:, c * CH:(c + 1) * CH],
                        s2[k - NZ2][:].rearrange("p (jb m) -> p jb m", m=M)
                        [:, :, mb * 128:(mb + 1) * 128],
                        m2s[k][:].rearrange("p (jb t) -> p jb t", t=T)
                        [:, :, c * CH:(c + 1) * CH],
                        start=(k == NZ2), stop=False, perf_mode=DR)
            for c in range(N_CH):
                nc.tensor.matmul(
                    ps2[:, c * CH:(c + 1) * CH],
                    sg2[:].rearrange("p (jb m) -> p jb m", m=M)
                    [:, :, mb * 128:(mb + 1) * 128],
                    spk3[:, :, c * CH:(c + 1) * CH],
                    start=False, stop=(c == N_CH - 1), perf_mode=DR)
            nc.scalar.activation(partial[mb][:], ps2[:], Act.Identity,
                                 bias=bo[:, mb:mb + 1], scale=-1.0 / N)
            nc.sync.dma_start(cin[mb * 128:(mb + 1) * 128, :],
                              partial[mb][:])
        nc.gpsimd.collective_compute(
            "AllToAll", mybir.AluOpType.bypass,
            replica_groups=[list(range(N_CORES))],
            ins=[cin.opt()], outs=[cout.opt()])
        rx = cpool.tile([MS, N_CORES * T], f16, tag="rx", name="rx")
        acc = cpool.tile([MS, T], f16, tag="acc", name="acc")
        T2 = T // 2
        for h in range(2):
            hs = h * T2
            for i in range(N_CORES):
                nc.sync.dma_start(
                    rx[:, i * T + hs:i * T + hs + T2],
                    cout[i * MS:(i + 1) * MS, hs:hs + T2])
            nc.vector.tensor_tensor(acc[:, hs:hs + T2], rx[:, hs:hs + T2],
                                    rx[:, T + hs:T + hs + T2],
                                    op=mybir.AluOpType.add)
            for i in range(2, N_CORES):
                nc.vector.tensor_tensor(acc[:, hs:hs + T2],
                                        acc[:, hs:hs + T2],
                                        rx[:, i * T + hs:i * T + hs + T2],
                                        op=mybir.AluOpType.add)
            nc.sync.dma_start(out_d[:, hs:hs + T2], acc[:, hs:hs + T2])

    nc.compile()
    return nc


def _quant_shared(W, nblk, dt, clamp_nonpos_below=None):
    """W [n_out, n_red] -> per-partition-index levels shared across the
    n_red/128 blocks. Levels sorted ascending. Returns lv [128, K] (f32,
    dt-representable), idx [n_out, n_red], Wq [n_out, n_red]."""
    n_out, n_red = W.shape
    Wp = W.reshape(n_out, nblk, 128)
    qs = (np.arange(K, dtype=np.float64) + 0.5) / K
    lv = np.quantile(Wp.transpose(2, 0, 1).reshape(128, -1), qs, axis=1).T
    if clamp_nonpos_below is not None:
        lv[:, :clamp_nonpos_below] = np.minimum(lv[:, :clamp_nonpos_below], 0)
    lv = lv.astype(dt).astype(np.float32)                    # [128, K]
    if clamp_nonpos_below is not None:
        lv[:, :clamp_nonpos_below] = np.minimum(lv[:, :clamp_nonpos_below], 0)
    idx = np.abs(Wp[:, :, :, None]
                 - lv[None, None, :, :]).argmin(axis=3)      # [n_out,blk,128]
    Wq = lv[np.arange(128)[None, None, :], idx]
    return lv, idx.reshape(n_out, n_red), Wq.reshape(n_out, n_red)


def prep_inputs(x, W_in, b_in, W_out, b_out, v_th, n_cores=N_CORES):
    """Host-side prep: pack/transpose + weight quantization per core."""
    import ml_dtypes

    f8 = ml_dtypes.float8_e4m3

    x2 = np.asarray(x, np.float32).reshape(T, D)
    x8 = np.ascontiguousarray(x2.T).astype(f8)               # [D, T]
    xp = np.ascontiguousarray(
        x8.reshape(N_DBLK, 128, T).transpose(1, 0, 2).reshape(128, N_DBLK * T))

    # L1 quantization is global (same W_in for every core).
    lv1, idx1, Wq1 = _quant_shared(np.asarray(W_in, np.float32), N_DBLK, f8)
    th1p = np.ascontiguousarray(lv1)
    th1n = np.ascontiguousarray(-lv1)
    # eta: +1 for relu-form (ACT) levels, -1 for max-form (DVE) levels
    eta1 = np.where(np.isin(np.arange(K), list(ACT1_KS)), 1.0, -1.0)
    etaw1 = (eta1[idx1] * Wq1).astype(np.float64).sum(axis=1)  # [N]
    beta_full = (SURR_BETA * (np.asarray(b_in, np.float64)
                              - np.asarray(v_th, np.float64)
                              - etaw1 / D)).astype(np.float32)

    in_maps = []
    for c in range(n_cores):
        sl = slice(c * JC, (c + 1) * JC)
        idx1_loc = idx1[sl, :]                               # [JC, D]
        # s1 rows (k*2+dbp)*128+p, cols two*JC + j
        s1 = np.zeros((K, 2, 128, 2 * JC), np.float32)
        for db in range(N_DBLK):
            dbp, two = divmod(db, 2)
            blk = idx1_loc[:, db * 128:(db + 1) * 128].T     # [128(p), JC]
            for k in range(K):
                s1[k, dbp, :, two * JC:(two + 1) * JC] = 2.0 * (blk == k)
        s1 = s1.reshape(K * 2 * 128, 2 * JC).astype(f8)

        beta = np.ascontiguousarray(
            beta_full[sl].reshape(N_JBLK, 128).T).astype(np.float32)

        # L2: quantize this core's W_out column slice. Levels 0..NZ2-1 are
        # clamped <= 0 and handled by the free zero form (spikes >= 0).
        W2c = np.asarray(W_out, np.float32)[:, sl]           # [M, JC]
        lv2, idx2, Wq2 = _quant_shared(W2c, N_JBLK, f8,
                                       clamp_nonpos_below=NZ2)
        th2p = np.ascontiguousarray(lv2)
        # s2 rows (k-NZ2)*128+p, cols jb*M + m  (positive levels only)
        s2 = np.zeros((K - NZ2, 128, N_JBLK * M), np.float32)
        sg2 = np.zeros((128, N_JBLK * M), np.float32)
        for jb in range(N_JBLK):
            blk = idx2[:, jb * 128:(jb + 1) * 128]           # [M, 128(p)]
            for k in range(NZ2, K):
                s2[k - NZ2, :, jb * M:(jb + 1) * M] = 2.0 * (blk == k).T
            sg2[:, jb * M:(jb + 1) * M] = np.where(
                (blk < NZ2) | np.isin(blk, list(ACT2_KS)), 1.0, -1.0).T
        s2 = s2.reshape((K - NZ2) * 128, N_JBLK * M).astype(f8)
        sg2 = sg2.astype(f8)

        # wq coefficient is -1 for both max and zero forms
        etaw2 = (-Wq2).astype(np.float64).sum(axis=1)        # [M]
        bo_full = (np.asarray(b_out, np.float64) / n_cores
                   - etaw2 / N).astype(np.float32)
        bo = np.ascontiguousarray(
            bo_full.reshape(N_MBLK, 128).T).astype(np.float32)

        in_maps.append({
            "xp": xp, "th1p": th1p, "th1n": th1n, "s1": s1, "beta": beta,
            "th2p": th2p, "s2": s2, "sg2": sg2, "bo": bo,
        })
    return in_maps


_NC_CACHE = {}


def _get_nc():
    if "nc" not in _NC_CACHE:
        _NC_CACHE["nc"] = build_kernel()
    return _NC_CACHE["nc"]


def run_on_hw(inputs, trace=False, tmpdir=None):
    """Run on the 8 NeuronCores; returns (full_output, BassKernelResults)."""
    from concourse.bass_utils import run_bass_kernel_spmd

    nc = _get_nc()
    in_maps = prep_inputs(**inputs)
    res = run_bass_kernel_spmd(nc, in_maps, core_ids=list(range(N_CORES)),
                               trace=trace, tmpdir=tmpdir)
    B, S, D_model = inputs["x"].shape
    full = np.empty((M, T), np.float32)
    for c in range(N_CORES):
        full[c * MS:(c + 1) * MS, :] = \
            np.asarray(res.results[c]["out"], np.float32)
    out = np.ascontiguousarray(full.T).reshape(B, S, D_model)
    return out.astype(np.float32), res


def kernel(x, W_in, b_in, W_out, b_out, v_th):
    out, _ = run_on_hw(dict(x=x, W_in=W_in, b_in=b_in, W_out=W_out,
                            b_out=b_out, v_th=v_th))
    return out
